# revision 3
# baseline (speedup 1.0000x reference)
"""Cross-parent attention kernel for Trainium2 (8 NeuronCores, SPMD).

Problem (hardcoded from spec): B=4, T=64, Nf=Np=384, C=128, h=2, dh=64.
  q = q_in @ Wq.T + bq ; k/v likewise from kv_in
  per (b,t,head): attn = softmax(q k^T / sqrt(dh)) ; out_h = attn @ v
  out = concat_heads @ Wo.T + bo

Sharding: data-parallel over the 256 (b,t) pairs -> 32 pairs per core.

Per-core kernel design (layouts keep the contraction dim on SBUF
partitions and every elementwise op partition-aligned):
  - x tiles are PE-transposed (f32r) to put channels on partitions.
  - qT/kT via weight-stationary f32r matmuls (N=384 -> 1 cyc/row).
  - v is produced token-major in bf16; a tiny rank-1 matmul writes "ones"
    columns next to it so attn @ [v|1] also yields the softmax
    denominator row (no-max softmax: scores ~ N(0,1), fp32-safe).
  - exp(scoresT) on ACT straight out of PSUM (1/sqrt(dh) folded into Wq).
  - denominator reciprocal (DVE) -> K=1 broadcast matmul -> normalize.
  - final projection split-K over heads, bf16, accumulated in PSUM.
"""

import numpy as np

B, T, NF, C = 4, 64, 384, 128
H, DH = 2, 64
NCORES = 8
PAIRS = B * T  # 256
PER_CORE = PAIRS // NCORES  # 32
SCALE = 1.0 / np.sqrt(DH)  # 0.125

_CACHE = {}


def _build(has_bias):
    import concourse.bacc as bacc
    import concourse.mybir as mybir
    from concourse.tile import TileContext

    F32 = mybir.dt.float32
    F32R = mybir.dt.float32r
    BF16 = mybir.dt.bfloat16
    AF = mybir.ActivationFunctionType
    MUL = mybir.AluOpType.mult

    nc = bacc.Bacc()

    xq_d = nc.declare_dram_parameter("xq", [PER_CORE, NF, C], F32R, isOutput=False)
    xkv_d = nc.declare_dram_parameter("xkv", [PER_CORE, NF, C], F32R, isOutput=False)
    wqt_d = nc.declare_dram_parameter("wqt", [C, C], F32R, isOutput=False)
    wkt_d = nc.declare_dram_parameter("wkt", [C, C], F32R, isOutput=False)
    wvt_d = nc.declare_dram_parameter("wvt", [C, C], BF16, isOutput=False)
    wot0_d = nc.declare_dram_parameter("wot0", [DH, C], BF16, isOutput=False)
    wot1_d = nc.declare_dram_parameter("wot1", [DH, C], BF16, isOutput=False)
    ident_d = nc.declare_dram_parameter("ident", [C, C], F32R, isOutput=False)
    ones64_d = nc.declare_dram_parameter("ones64", [1, DH], F32R, isOutput=False)
    ones1_d = nc.declare_dram_parameter("ones1", [1, C], BF16, isOutput=False)
    ones2_d = nc.declare_dram_parameter("ones2", [1, 2], BF16, isOutput=False)
    if has_bias:
        bq_d = nc.declare_dram_parameter("bqc", [C, 1], F32, isOutput=False)
        bk_d = nc.declare_dram_parameter("bkc", [C, 1], F32, isOutput=False)
        bop_d = nc.declare_dram_parameter("bop", [1, C], BF16, isOutput=False)
    out_d = nc.declare_dram_parameter("out", [PER_CORE, NF, C], F32, isOutput=True)

    with TileContext(nc) as tc:
        with (
            tc.tile_pool(name="static", bufs=1) as stat,
            tc.tile_pool(name="xin", bufs=3) as xin,
            tc.tile_pool(name="xt", bufs=2) as xtp,
            tc.tile_pool(name="qk", bufs=2) as qkp,
            tc.tile_pool(name="vexp", bufs=2) as vexp,
            tc.tile_pool(name="small", bufs=2) as smallp,
            tc.tile_pool(name="outp", bufs=3) as outp,
            tc.tile_pool(name="ps_tp", bufs=2, space="PSUM") as tps,
            tc.tile_pool(name="ps_pj", bufs=2, space="PSUM") as pjp,
            tc.tile_pool(name="ps_sc", bufs=2, space="PSUM") as scp,
            tc.tile_pool(name="ps_at", bufs=2, space="PSUM") as atp,
        ):
            # ---- static loads ----
            wqt = stat.tile([C, C], F32R, tag="wqt")
            wkt = stat.tile([C, C], F32R, tag="wkt")
            wvt = stat.tile([C, C], BF16, tag="wvt")
            wot0 = stat.tile([DH, C], BF16, tag="wot0")
            wot1 = stat.tile([DH, C], BF16, tag="wot1")
            ident = stat.tile([C, C], F32R, tag="ident")
            ones64 = stat.tile([DH + 1, DH], F32R, tag="ones64")
            ones1 = stat.tile([1, C], BF16, tag="ones1")
            ones2 = stat.tile([1, 2], BF16, tag="ones2")
            nc.sync.dma_start(out=wqt[:], in_=wqt_d[:])
            nc.sync.dma_start(out=wkt[:], in_=wkt_d[:])
            nc.sync.dma_start(out=wvt[:], in_=wvt_d[:])
            nc.sync.dma_start(out=wot0[:], in_=wot0_d[:])
            nc.sync.dma_start(out=wot1[:], in_=wot1_d[:])
            nc.sync.dma_start(out=ident[:], in_=ident_d[:])
            nc.sync.dma_start(out=ones64[DH : DH + 1, :], in_=ones64_d[:])
            nc.sync.dma_start(out=ones1[:], in_=ones1_d[:])
            nc.sync.dma_start(out=ones2[:], in_=ones2_d[:])
            if has_bias:
                bqc = stat.tile([C, 1], F32, tag="bqc")
                bkc = stat.tile([C, 1], F32, tag="bkc")
                bop = stat.tile([1, C], BF16, tag="bop")
                nc.sync.dma_start(out=bqc[:], in_=bq_d[:])
                nc.sync.dma_start(out=bkc[:], in_=bk_d[:])
                nc.sync.dma_start(out=bop[:], in_=bop_d[:])

            for n in range(PER_CORE):
                # ---- load x (token-major), [p, (chunk c)] view ----
                xq = xin.tile([128, 3 * C], F32R, tag="xq")
                xkv = xin.tile([128, 3 * C], F32R, tag="xkv")
                nc.sync.dma_start(
                    out=xq[:].rearrange("p (a c) -> p a c", a=3),
                    in_=xq_d[n].rearrange("(a p) c -> p a c", p=128),
                )
                nc.sync.dma_start(
                    out=xkv[:].rearrange("p (a c) -> p a c", a=3),
                    in_=xkv_d[n].rearrange("(a p) c -> p a c", p=128),
                )

                # ---- transpose x -> channels on partitions ----
                xtq = xtp.tile([C, NF], F32R, tag="xtq")
                xtkv = xtp.tile([C, NF], F32R, tag="xtkv")
                xtkv_bf = xtp.tile([C, NF], BF16, tag="xtkvbf")
                for a in range(3):
                    sl = slice(a * 128, (a + 1) * 128)
                    tp = tps.tile([128, 128], F32R, tag="tp")
                    nc.tensor.transpose(tp[:], xq[:, sl], ident[:])
                    nc.scalar.copy(xtq[:, sl], tp[:])
                for a in range(3):
                    sl = slice(a * 128, (a + 1) * 128)
                    tp = tps.tile([128, 128], F32R, tag="tp")
                    nc.tensor.transpose(tp[:], xkv[:, sl], ident[:])
                    nc.scalar.copy(xtkv[:, sl], tp[:])
                    nc.vector.tensor_copy(xtkv_bf[:, sl], tp[:].bitcast(F32))

                # ---- q/k projections (transposed layout) ----
                qps = pjp.tile([C, NF], F32, tag="pj")
                nc.tensor.matmul(qps[:], wqt[:], xtq[:], start=True, stop=True)
                qt = qkp.tile([C, NF], F32R, tag="qt")
                if has_bias:
                    with nc.allow_low_precision(reason="f32r rounding of qT"):
                        nc.scalar.activation(
                            qt[:], qps[:], AF.Identity, bias=bqc[:], scale=1.0
                        )
                else:
                    nc.vector.tensor_copy(qt[:], qps[:])

                kps = pjp.tile([C, NF], F32, tag="pj")
                nc.tensor.matmul(kps[:], wkt[:], xtkv[:], start=True, stop=True)
                kt = qkp.tile([C, NF], F32R, tag="kt")
                if has_bias:
                    with nc.allow_low_precision(reason="f32r rounding of kT"):
                        nc.scalar.activation(
                            kt[:], kps[:], AF.Identity, bias=bkc[:], scale=1.0
                        )
                else:
                    nc.vector.tensor_copy(kt[:], kps[:])

                # ---- v projection, token-major bf16, with ones cols ----
                # vsb layout per chunk a: cols [a*130, a*130+130) =
                #   [v_h0(64) | 1 | v_h1(64) | 1]
                vsb = vexp.tile([128, 3 * 130], BF16, tag="v")
                for a in range(3):
                    sl = slice(a * 128, (a + 1) * 128)
                    vps = pjp.tile([128, 130], F32, tag="pj")
                    nc.tensor.matmul(
                        vps[:, 0:128], xtkv_bf[:, sl], wvt[:], start=True, stop=True
                    )
                    nc.tensor.matmul(
                        vps[:, 128:130], ones1[:], ones2[:], start=True, stop=True
                    )
                    vt_v = vsb[:, a * 130 : (a + 1) * 130].rearrange(
                        "p (h x) -> p h x", h=2
                    )
                    nc.vector.tensor_copy(
                        vt_v[:, :, 0:DH],
                        vps[:, 0:128].rearrange("p (h d) -> p h d", h=2),
                    )
                    nc.vector.tensor_copy(
                        vt_v[:, :, DH : DH + 1],
                        vps[:, 128:130].rearrange("p (h o) -> p h o", h=2),
                    )

                # ---- per-head attention ----
                onorm = outp.tile([DH, 2 * NF], BF16, tag="on")
                for h in range(H):
                    hs = h * DH
                    esb = vexp.tile([128, 3 * NF], BF16, tag="exp")
                    for a in range(3):
                        scps = scp.tile([128, NF], F32, tag="sc")
                        nc.tensor.matmul(
                            scps[:],
                            kt[hs : hs + DH, a * 128 : (a + 1) * 128],
                            qt[hs : hs + DH, :],
                            start=True,
                            stop=True,
                        )
                        nc.scalar.activation(
                            esb[:, a * NF : (a + 1) * NF], scps[:], AF.Exp, scale=1.0
                        )

                    # attn @ [v | 1]: accumulate over j chunks
                    at = atp.tile([DH + 1, NF], F32, tag="at")
                    for a in range(3):
                        nc.tensor.matmul(
                            at[:],
                            vsb[:, a * 130 + h * 65 : a * 130 + (h + 1) * 65],
                            esb[:, a * NF : (a + 1) * NF],
                            start=(a == 0),
                            stop=(a == 2),
                        )

                    # reciprocal of denominator row (partition 64)
                    rc = smallp.tile([DH + 1, NF], F32R, tag="rc")
                    with nc.allow_low_precision(reason="softmax denom recip"):
                        nc.vector.reciprocal(rc[DH : DH + 1, :], at[DH : DH + 1, :])
                    # broadcast to 64 partitions (K=1 matmul from partition 64)
                    bc = scp.tile([DH, NF], F32, tag="sc")
                    nc.tensor.matmul(
                        bc[:],
                        ones64[DH : DH + 1, :],
                        rc[DH : DH + 1, :],
                        start=True,
                        stop=True,
                    )
                    bcs = smallp.tile([DH, NF], F32, tag="bcs")
                    nc.scalar.copy(bcs[:], bc[:])
                    nc.vector.tensor_tensor(
                        onorm[:, h * NF : (h + 1) * NF], at[0:DH, :], bcs[:], op=MUL
                    )

                # ---- final projection: split-K over heads, bf16 ----
                fout = outp.tile([128, 3 * C], F32, tag="fout")
                for a in range(3):
                    sl = slice(a * 128, (a + 1) * 128)
                    fps = tps.tile([128, 128], F32, tag="tp")
                    nc.tensor.matmul(
                        fps[:], onorm[:, 0 * NF + a * 128 : 0 * NF + (a + 1) * 128],
                        wot0[:], start=True, stop=False,
                    )
                    nc.tensor.matmul(
                        fps[:], onorm[:, 1 * NF + a * 128 : 1 * NF + (a + 1) * 128],
                        wot1[:], start=False, stop=not has_bias,
                    )
                    if has_bias:
                        nc.tensor.matmul(
                            fps[:], ones1[:], bop[:], start=False, stop=True
                        )
                    nc.vector.tensor_copy(fout[:, sl], fps[:])
                nc.sync.dma_start(
                    out=out_d[n].rearrange("(a p) c -> p a c", p=128),
                    in_=fout[:].rearrange("p (a c) -> p a c", a=3),
                )

    nc.finalize()
    return nc


def _get_nc(has_bias):
    key = ("nc", has_bias)
    if key not in _CACHE:
        _CACHE[key] = _build(has_bias)
    return _CACHE[key]


def kernel(q_in, kv_in, Wq, bq, Wk, bk, Wv, bv, Wo, bo):
    import ml_dtypes
    from concourse.bass_utils import run_bass_kernel_spmd

    q_in = np.asarray(q_in, dtype=np.float32)
    kv_in = np.asarray(kv_in, dtype=np.float32)
    Wq = np.asarray(Wq, dtype=np.float32)
    Wk = np.asarray(Wk, dtype=np.float32)
    Wv = np.asarray(Wv, dtype=np.float32)
    Wo = np.asarray(Wo, dtype=np.float32)
    bq = np.asarray(bq, dtype=np.float32)
    bk = np.asarray(bk, dtype=np.float32)
    bv = np.asarray(bv, dtype=np.float32)
    bo = np.asarray(bo, dtype=np.float32)

    bf16 = ml_dtypes.bfloat16
    # fold 1/sqrt(dh) into Wq/bq; fold bv through softmax (rows sum to 1)
    # and Wo into the output bias: out = attn@(v0 + 1 bv) @ Wo.T + bo
    #                                  = attn@v0 @ Wo.T + 1 (Wo bv + bo).
    wqt = np.ascontiguousarray(Wq.T) * np.float32(SCALE)
    wkt = np.ascontiguousarray(Wk.T)
    wvt = np.ascontiguousarray(Wv.T).astype(bf16)
    wot = Wo.T  # [c, c']
    wot0 = np.ascontiguousarray(wot[0:DH, :]).astype(bf16)
    wot1 = np.ascontiguousarray(wot[DH:C, :]).astype(bf16)
    bqs = (bq * np.float32(SCALE)).reshape(C, 1)
    bks = bk.reshape(C, 1)
    bop = (Wo @ bv + bo).reshape(1, C)
    has_bias = bool(np.any(bqs) or np.any(bks) or np.any(bop))

    nc = _get_nc(has_bias)

    qf = q_in.reshape(PAIRS, NF, C)
    kf = kv_in.reshape(PAIRS, NF, C)

    common = {
        "wqt": wqt,
        "wkt": wkt,
        "wvt": wvt,
        "wot0": wot0,
        "wot1": wot1,
        "ident": np.eye(C, dtype=np.float32),
        "ones64": np.ones((1, DH), dtype=np.float32),
        "ones1": np.ones((1, C), dtype=bf16),
        "ones2": np.ones((1, 2), dtype=bf16),
    }
    if has_bias:
        common["bqc"] = bqs
        common["bkc"] = bks
        common["bop"] = bop.astype(bf16)

    in_maps = []
    for i in range(NCORES):
        m = dict(common)
        m["xq"] = np.ascontiguousarray(qf[i * PER_CORE : (i + 1) * PER_CORE])
        m["xkv"] = np.ascontiguousarray(kf[i * PER_CORE : (i + 1) * PER_CORE])
        in_maps.append(m)

    res = run_bass_kernel_spmd(nc, in_maps, list(range(NCORES)))
    out = np.concatenate([res.results[i]["out"] for i in range(NCORES)], axis=0)
    return out.reshape(B, T, NF, C)


# revision 9
# speedup vs baseline: 11.8264x; 11.8264x over previous
"""Cross-parent attention kernel for Trainium2 (8 NeuronCores, SPMD).

Problem (hardcoded from spec): B=4, T=64, Nf=Np=384, C=128, h=2, dh=64.
  q = q_in @ Wq.T + bq ; k/v likewise from kv_in
  per (b,t,head): attn = softmax(q k^T / sqrt(dh)) ; out_h = attn @ v
  out = concat_heads @ Wo.T + bo

Sharding: data-parallel over the 256 (b,t) pairs -> 32 pairs per core.

Per-core design notes:
  - x tiles PE-transposed (f32r) into ONE [128,384] PSUM tile per input,
    so each needs a single PSUM->SBUF copy (bf16).
  - q/k projections in bf16 (N=384, 1 cyc/row); their PSUM results are
    rounded to f32r so the scores matmuls keep ~13-bit mantissa.
  - v token-major bf16 with rank-1 "ones" columns in the same PSUM tile;
    attn @ [v|1] gives the softmax denominator row for free
    (no-max softmax: scores ~ N(0,1), fp32-safe).
  - exp(scoresT) on ACT straight out of PSUM.
  - denominator: DVE reciprocal -> gpsimd partition_broadcast (Pool is
    otherwise idle) -> DVE multiply to normalized bf16.
  - final projection split-K over heads (bf16) into one PSUM tile.
  - emission is software-pipelined across bt (stage lag) so every engine
    FIFO holds independent work from several iterations.
"""

import numpy as np

B, T, NF, C = 4, 64, 384, 128
H, DH = 2, 64
NCORES = 8
PAIRS = B * T  # 256
PER_CORE = PAIRS // NCORES  # 32
SCALE = 1.0 / np.sqrt(DH)  # 0.125

_CACHE = {}


def _build(has_bias, n_pairs=PER_CORE):
    import concourse.bacc as bacc
    import concourse.mybir as mybir
    from concourse.tile import TileContext

    F32 = mybir.dt.float32
    F32R = mybir.dt.float32r
    BF16 = mybir.dt.bfloat16
    AF = mybir.ActivationFunctionType
    MUL = mybir.AluOpType.mult

    nc = bacc.Bacc()

    xq_d = nc.declare_dram_parameter("xq", [PER_CORE, NF, C], F32R, isOutput=False)
    xkv_d = nc.declare_dram_parameter("xkv", [PER_CORE, NF, C], F32R, isOutput=False)
    wqt_d = nc.declare_dram_parameter("wqt", [C, C], BF16, isOutput=False)
    wkt_d = nc.declare_dram_parameter("wkt", [C, C], BF16, isOutput=False)
    wvt_d = nc.declare_dram_parameter("wvt", [C, C], BF16, isOutput=False)
    wot0_d = nc.declare_dram_parameter("wot0", [DH, C], BF16, isOutput=False)
    wot1_d = nc.declare_dram_parameter("wot1", [DH, C], BF16, isOutput=False)
    ident_d = nc.declare_dram_parameter("ident", [C, C], F32R, isOutput=False)
    ones64_d = nc.declare_dram_parameter("ones64", [1, DH], F32R, isOutput=False)
    ones1_d = nc.declare_dram_parameter("ones1", [1, C], BF16, isOutput=False)
    ones2_d = nc.declare_dram_parameter("ones2", [1, 2], BF16, isOutput=False)
    if has_bias:
        bq_d = nc.declare_dram_parameter("bqc", [C, 1], F32, isOutput=False)
        bk_d = nc.declare_dram_parameter("bkc", [C, 1], F32, isOutput=False)
        bop_d = nc.declare_dram_parameter("bop", [1, C], BF16, isOutput=False)
    out_d = nc.declare_dram_parameter("out", [PER_CORE, NF, C], F32, isOutput=True)

    with TileContext(nc) as tc:
        with (
            tc.tile_pool(name="static", bufs=1) as stat,
            tc.tile_pool(name="xin", bufs=3) as xin,
            tc.tile_pool(name="xt", bufs=3) as xtp,
            tc.tile_pool(name="qk", bufs=3) as qkp,
            tc.tile_pool(name="vexp", bufs=3) as vexp,
            tc.tile_pool(name="small", bufs=3) as smallp,
            tc.tile_pool(name="outp", bufs=3) as outp,
            tc.tile_pool(name="ps_tp", bufs=2, space="PSUM") as tps,
            tc.tile_pool(name="ps_pj", bufs=2, space="PSUM") as pjp,
            tc.tile_pool(name="ps_sc", bufs=2, space="PSUM") as scp,
            tc.tile_pool(name="ps_at", bufs=2, space="PSUM") as atp,
        ):
            # ---- static loads ----
            wqt = stat.tile([C, C], BF16, tag="wqt")
            wkt = stat.tile([C, C], BF16, tag="wkt")
            wvt = stat.tile([C, C], BF16, tag="wvt")
            wot0 = stat.tile([DH, C], BF16, tag="wot0")
            wot1 = stat.tile([DH, C], BF16, tag="wot1")
            ident = stat.tile([C, C], F32R, tag="ident")
            ones64 = stat.tile([DH + 1, DH], F32R, tag="ones64")
            ones1 = stat.tile([1, C], BF16, tag="ones1")
            ones2 = stat.tile([1, 2], BF16, tag="ones2")
            nc.sync.dma_start(out=wqt[:], in_=wqt_d[:])
            nc.sync.dma_start(out=wkt[:], in_=wkt_d[:])
            nc.sync.dma_start(out=wvt[:], in_=wvt_d[:])
            nc.sync.dma_start(out=wot0[:], in_=wot0_d[:])
            nc.sync.dma_start(out=wot1[:], in_=wot1_d[:])
            nc.sync.dma_start(out=ident[:], in_=ident_d[:])
            nc.sync.dma_start(out=ones64[DH : DH + 1, :], in_=ones64_d[:])
            nc.sync.dma_start(out=ones1[:], in_=ones1_d[:])
            nc.sync.dma_start(out=ones2[:], in_=ones2_d[:])
            if has_bias:
                bqc = stat.tile([C, 1], F32, tag="bqc")
                bkc = stat.tile([C, 1], F32, tag="bkc")
                bop = stat.tile([1, C], BF16, tag="bop")
                nc.sync.dma_start(out=bqc[:], in_=bq_d[:])
                nc.sync.dma_start(out=bkc[:], in_=bk_d[:])
                nc.sync.dma_start(out=bop[:], in_=bop_d[:])

            # per-iteration state handed between pipeline stages
            st = [dict() for _ in range(n_pairs)]

            def s0_load(n):
                s = st[n]
                s["xq"] = xin.tile([128, 3 * C], F32R, tag="xq", name=f"xq{n}")
                s["xkv"] = xin.tile([128, 3 * C], F32R, tag="xkv", name=f"xkv{n}")
                nc.sync.dma_start(
                    out=s["xq"][:].rearrange("p (a c) -> p a c", a=3),
                    in_=xq_d[n].rearrange("(a p) c -> p a c", p=128),
                )
                nc.sync.dma_start(
                    out=s["xkv"][:].rearrange("p (a c) -> p a c", a=3),
                    in_=xkv_d[n].rearrange("(a p) c -> p a c", p=128),
                )

            def s1_transpose(n):
                s = st[n]
                tq = tps.tile([C, NF], F32R, tag="tp", name=f"tq{n}")
                tkv = tps.tile([C, NF], F32R, tag="tp", name=f"tkv{n}")
                for a in range(3):
                    sl = slice(a * 128, (a + 1) * 128)
                    nc.tensor.transpose(tq[:, sl], s["xq"][:, sl], ident[:])
                    nc.tensor.transpose(tkv[:, sl], s["xkv"][:, sl], ident[:])
                s["xtq"] = xtp.tile([C, NF], BF16, tag="xtq", name=f"xtq{n}")
                s["xtkv"] = xtp.tile([C, NF], BF16, tag="xtkv", name=f"xtkv{n}")
                nc.vector.tensor_copy(s["xtq"][:], tq[:].bitcast(F32))
                nc.scalar.copy(s["xtkv"][:], tkv[:].bitcast(F32))

            def s2_proj(n):
                s = st[n]
                qps = pjp.tile([C, NF], F32, tag="pj", name=f"qps{n}")
                nc.tensor.matmul(qps[:], wqt[:], s["xtq"][:], start=True, stop=True)
                s["qt"] = qkp.tile([C, NF], F32R, tag="qt", name=f"qt{n}")
                if has_bias:
                    with nc.allow_low_precision(reason="f32r rounding of qT"):
                        nc.scalar.activation(
                            s["qt"][:], qps[:], AF.Identity, bias=bqc[:], scale=1.0
                        )
                else:
                    nc.vector.tensor_copy(s["qt"][:], qps[:])

                kps = pjp.tile([C, NF], F32, tag="pj", name=f"kps{n}")
                nc.tensor.matmul(kps[:], wkt[:], s["xtkv"][:], start=True, stop=True)
                s["kt"] = qkp.tile([C, NF], F32R, tag="kt", name=f"kt{n}")
                if has_bias:
                    with nc.allow_low_precision(reason="f32r rounding of kT"):
                        nc.scalar.activation(
                            s["kt"][:], kps[:], AF.Identity, bias=bkc[:], scale=1.0
                        )
                else:
                    nc.vector.tensor_copy(s["kt"][:], kps[:])

                # v: 3 chunk matmuls + rank-1 ones into one [128, 390] psum
                vps = pjp.tile([128, 3 * 130], F32, tag="pj", name=f"vps{n}")
                for a in range(3):
                    o = a * 130
                    nc.tensor.matmul(
                        vps[:, o : o + 128],
                        s["xtkv"][:, a * 128 : (a + 1) * 128],
                        wvt[:],
                        start=True,
                        stop=True,
                    )
                    nc.tensor.matmul(
                        vps[:, o + 128 : o + 130], ones1[:], ones2[:],
                        start=True, stop=True,
                    )
                s["v"] = vexp.tile([128, 3 * 130], BF16, tag="v", name=f"v{n}")
                # [p, a, h*65 + d] <- psum[p, a, h*64 + d]
                nc.vector.tensor_copy(
                    s["v"][:]
                    .rearrange("p (a x) -> p a x", a=3)[:, :, 0:130]
                    .rearrange("p a (h x) -> p a h x", h=2)[:, :, :, 0:DH],
                    vps[:]
                    .rearrange("p (a x) -> p a x", a=3)[:, :, 0:128]
                    .rearrange("p a (h d) -> p a h d", h=2),
                )
                nc.vector.tensor_copy(
                    s["v"][:]
                    .rearrange("p (a x) -> p a x", a=3)[:, :, 0:130]
                    .rearrange("p a (h x) -> p a h x", h=2)[:, :, :, DH : DH + 1],
                    vps[:]
                    .rearrange("p (a x) -> p a x", a=3)[:, :, 128:130]
                    .rearrange("p a (h o) -> p a h o", h=2),
                )

            def s3_attention(n):
                s = st[n]
                s["osb"] = outp.tile([DH, 2 * NF], BF16, tag="on", name=f"on{n}")
                atsb = [None, None]
                for h in range(H):
                    hs = h * DH
                    esb = vexp.tile([128, 3 * NF], BF16, tag="exp", name=f"e{n}_{h}")
                    for a in range(3):
                        scps = scp.tile([128, NF], F32, tag="sc", name=f"sc{n}_{h}{a}")
                        nc.tensor.matmul(
                            scps[:],
                            s["kt"][hs : hs + DH, a * 128 : (a + 1) * 128],
                            s["qt"][hs : hs + DH, :],
                            start=True,
                            stop=True,
                        )
                        nc.scalar.activation(
                            esb[:, a * NF : (a + 1) * NF], scps[:], AF.Exp, scale=1.0
                        )
                    at = atp.tile([DH + 1, NF], F32, tag="at", name=f"at{n}_{h}")
                    for a in range(3):
                        nc.tensor.matmul(
                            at[:],
                            s["v"][:, a * 130 + h * 65 : a * 130 + (h + 1) * 65],
                            esb[:, a * NF : (a + 1) * NF],
                            start=(a == 0),
                            stop=(a == 2),
                        )
                    # free the PSUM accumulator: copy (ACT) and recip (DVE)
                    # both read it immediately and in parallel
                    un = smallp.tile([DH, NF], F32, tag="un", name=f"un{n}_{h}")
                    nc.scalar.copy(un[:], at[0:DH, :])
                    rc = smallp.tile([DH + 1, NF], F32R, tag="rc", name=f"rc{n}_{h}")
                    with nc.allow_low_precision(reason="softmax denom recip"):
                        nc.vector.reciprocal(rc[DH : DH + 1, :], at[DH : DH + 1, :])
                    atsb[h] = (un, rc)
                for h in range(H):
                    un, rc = atsb[h]
                    bc = scp.tile([DH, NF], F32, tag="sc", name=f"bc{n}_{h}")
                    nc.tensor.matmul(
                        bc[:], ones64[DH : DH + 1, :], rc[DH : DH + 1, :],
                        start=True, stop=True,
                    )
                    nc.vector.tensor_tensor(
                        s["osb"][:, h * NF : (h + 1) * NF], un[:], bc[:], op=MUL
                    )

            def s4_final(n):
                s = st[n]
                fps = tps.tile([128, 3 * C], F32, tag="tp", name=f"fps{n}")
                for a in range(3):
                    sl = slice(a * 128, (a + 1) * 128)
                    nc.tensor.matmul(
                        fps[:, sl],
                        s["osb"][:, 0 * NF + a * 128 : 0 * NF + (a + 1) * 128],
                        wot0[:],
                        start=True,
                        stop=False,
                    )
                    nc.tensor.matmul(
                        fps[:, sl],
                        s["osb"][:, 1 * NF + a * 128 : 1 * NF + (a + 1) * 128],
                        wot1[:],
                        start=False,
                        stop=not has_bias,
                    )
                    if has_bias:
                        nc.tensor.matmul(
                            fps[:, sl], ones1[:], bop[:], start=False, stop=True
                        )
                fout = outp.tile([128, 3 * C], F32, tag="fout", name=f"fo{n}")
                nc.vector.tensor_copy(fout[:], fps[:])
                nc.sync.dma_start(
                    out=out_d[n].rearrange("(a p) c -> p a c", p=128),
                    in_=fout[:].rearrange("p (a c) -> p a c", a=3),
                )
                st[n] = None  # release references

            # software-pipelined emission, oldest stage first
            stages = [s0_load, s1_transpose, s2_proj, s3_attention, s4_final]
            NS = len(stages)
            for step in range(n_pairs + NS - 1):
                for k in range(NS - 1, -1, -1):
                    i = step - k
                    if 0 <= i < n_pairs:
                        stages[k](i)

    nc.finalize()
    return nc


def _get_nc(has_bias, n_pairs=PER_CORE):
    key = ("nc", has_bias, n_pairs)
    if key not in _CACHE:
        _CACHE[key] = _build(has_bias, n_pairs)
    return _CACHE[key]


def kernel(q_in, kv_in, Wq, bq, Wk, bk, Wv, bv, Wo, bo):
    import ml_dtypes
    from concourse.bass_utils import run_bass_kernel_spmd

    q_in = np.asarray(q_in, dtype=np.float32)
    kv_in = np.asarray(kv_in, dtype=np.float32)
    Wq = np.asarray(Wq, dtype=np.float32)
    Wk = np.asarray(Wk, dtype=np.float32)
    Wv = np.asarray(Wv, dtype=np.float32)
    Wo = np.asarray(Wo, dtype=np.float32)
    bq = np.asarray(bq, dtype=np.float32)
    bk = np.asarray(bk, dtype=np.float32)
    bv = np.asarray(bv, dtype=np.float32)
    bo = np.asarray(bo, dtype=np.float32)

    bf16 = ml_dtypes.bfloat16
    # fold 1/sqrt(dh) into Wq/bq; fold bv through softmax (rows sum to 1)
    # and Wo into the output bias: out = attn@(v0 + 1 bv) @ Wo.T + bo
    #                                  = attn@v0 @ Wo.T + 1 (Wo bv + bo).
    wqt = (np.ascontiguousarray(Wq.T) * np.float32(SCALE)).astype(bf16)
    wkt = np.ascontiguousarray(Wk.T).astype(bf16)
    wvt = np.ascontiguousarray(Wv.T).astype(bf16)
    wot = Wo.T  # [c, c']
    wot0 = np.ascontiguousarray(wot[0:DH, :]).astype(bf16)
    wot1 = np.ascontiguousarray(wot[DH:C, :]).astype(bf16)
    bqs = (bq * np.float32(SCALE)).reshape(C, 1)
    bks = bk.reshape(C, 1)
    bop = (Wo @ bv + bo).reshape(1, C)
    has_bias = bool(np.any(bqs) or np.any(bks) or np.any(bop))

    nc = _get_nc(has_bias)

    qf = q_in.reshape(PAIRS, NF, C)
    kf = kv_in.reshape(PAIRS, NF, C)

    common = {
        "wqt": wqt,
        "wkt": wkt,
        "wvt": wvt,
        "wot0": wot0,
        "wot1": wot1,
        "ident": np.eye(C, dtype=np.float32),
        "ones64": np.ones((1, DH), dtype=np.float32),
        "ones1": np.ones((1, C), dtype=bf16),
        "ones2": np.ones((1, 2), dtype=bf16),
    }
    if has_bias:
        common["bqc"] = bqs
        common["bkc"] = bks
        common["bop"] = bop.astype(bf16)

    in_maps = []
    for i in range(NCORES):
        m = dict(common)
        m["xq"] = np.ascontiguousarray(qf[i * PER_CORE : (i + 1) * PER_CORE])
        m["xkv"] = np.ascontiguousarray(kf[i * PER_CORE : (i + 1) * PER_CORE])
        in_maps.append(m)

    res = run_bass_kernel_spmd(nc, in_maps, list(range(NCORES)))
    out = np.concatenate([res.results[i]["out"] for i in range(NCORES)], axis=0)
    return out.reshape(B, T, NF, C)


# revision 12
# speedup vs baseline: 103.3948x; 8.7427x over previous
"""Cross-parent attention kernel for Trainium2 (8 NeuronCores, SPMD).

Problem (hardcoded from spec): B=4, T=64, Nf=Np=384, C=128, h=2, dh=64.
  q = q_in @ Wq.T + bq ; k/v likewise from kv_in
  per (b,t,head): attn = softmax(q k^T / sqrt(dh)) ; out_h = attn @ v
  out = concat_heads @ Wo.T + bo

Sharding: data-parallel over the 256 (b,t) pairs -> 32 pairs per core.

Per-core design notes:
  - x tiles PE-transposed (f32r) into ONE [128,384] PSUM tile per input,
    so each needs a single PSUM->SBUF copy (bf16).
  - q/k projections in bf16 (N=384, 1 cyc/row); their PSUM results are
    rounded to f32r so the scores matmuls keep ~13-bit mantissa.
  - v token-major bf16 with rank-1 "ones" columns in the same PSUM tile;
    attn @ [v|1] gives the softmax denominator row for free
    (no-max softmax: scores ~ N(0,1), fp32-safe).
  - exp(scoresT) on ACT straight out of PSUM.
  - denominator: DVE reciprocal of the PSUM row -> K=1 PE matmul
    broadcast -> DVE multiply to normalized bf16 (one-PSUM-input rule).
  - final projection split-K over heads (bf16) into one PSUM tile.
  - emission is software-pipelined across bt (stage lag) so every engine
    FIFO holds independent work from several iterations.
"""

import numpy as np

B, T, NF, C = 4, 64, 384, 128
H, DH = 2, 64
NCORES = 8
PAIRS = B * T  # 256
PER_CORE = PAIRS // NCORES  # 32
SCALE = 1.0 / np.sqrt(DH)  # 0.125

_CACHE = {}


def _build(has_bias, n_pairs=PER_CORE, repeat=1):
    import concourse.bacc as bacc
    import concourse.mybir as mybir
    from concourse.tile import TileContext

    F32 = mybir.dt.float32
    F32R = mybir.dt.float32r
    BF16 = mybir.dt.bfloat16
    AF = mybir.ActivationFunctionType
    MUL = mybir.AluOpType.mult

    nc = bacc.Bacc()

    xq_d = nc.declare_dram_parameter("xq", [PER_CORE, NF, C], F32R, isOutput=False)
    xkv_d = nc.declare_dram_parameter("xkv", [PER_CORE, NF, C], F32R, isOutput=False)
    wqt_d = nc.declare_dram_parameter("wqt", [C, C], BF16, isOutput=False)
    wkt_d = nc.declare_dram_parameter("wkt", [C, C], BF16, isOutput=False)
    wvt_d = nc.declare_dram_parameter("wvt", [C, C], BF16, isOutput=False)
    wot0_d = nc.declare_dram_parameter("wot0", [DH, C], BF16, isOutput=False)
    wot1_d = nc.declare_dram_parameter("wot1", [DH, C], BF16, isOutput=False)
    ident_d = nc.declare_dram_parameter("ident", [C, C], F32R, isOutput=False)
    ones64_d = nc.declare_dram_parameter("ones64", [1, DH], F32R, isOutput=False)
    ones1_d = nc.declare_dram_parameter("ones1", [1, C], BF16, isOutput=False)
    ones2_d = nc.declare_dram_parameter("ones2", [1, 2], BF16, isOutput=False)
    if has_bias:
        bq_d = nc.declare_dram_parameter("bqc", [C, 1], F32, isOutput=False)
        bk_d = nc.declare_dram_parameter("bkc", [C, 1], F32, isOutput=False)
        bop_d = nc.declare_dram_parameter("bop", [1, C], BF16, isOutput=False)
    out_d = nc.declare_dram_parameter("out", [PER_CORE, NF, C], F32, isOutput=True)

    with TileContext(nc) as tc:
        with (
            tc.tile_pool(name="static", bufs=1) as stat,
            tc.tile_pool(name="xin", bufs=3) as xin,
            tc.tile_pool(name="xt", bufs=3) as xtp,
            tc.tile_pool(name="qk", bufs=3) as qkp,
            tc.tile_pool(name="vexp", bufs=3) as vexp,
            tc.tile_pool(name="small", bufs=3) as smallp,
            tc.tile_pool(name="outp", bufs=3) as outp,
            tc.tile_pool(name="ps_tp", bufs=2, space="PSUM") as tps,
            tc.tile_pool(name="ps_pj", bufs=2, space="PSUM") as pjp,
            tc.tile_pool(name="ps_sc", bufs=2, space="PSUM") as scp,
            tc.tile_pool(name="ps_at", bufs=2, space="PSUM") as atp,
        ):
            # ---- static loads ----
            wqt = stat.tile([C, C], BF16, tag="wqt")
            wkt = stat.tile([C, C], BF16, tag="wkt")
            wvt = stat.tile([C, C], BF16, tag="wvt")
            wot0 = stat.tile([DH, C], BF16, tag="wot0")
            wot1 = stat.tile([DH, C], BF16, tag="wot1")
            ident = stat.tile([C, C], F32R, tag="ident")
            ones64 = stat.tile([DH + 1, DH], F32R, tag="ones64")
            ones1 = stat.tile([1, C], BF16, tag="ones1")
            ones2 = stat.tile([1, 2], BF16, tag="ones2")
            nc.sync.dma_start(out=wqt[:], in_=wqt_d[:])
            nc.sync.dma_start(out=wkt[:], in_=wkt_d[:])
            nc.sync.dma_start(out=wvt[:], in_=wvt_d[:])
            nc.sync.dma_start(out=wot0[:], in_=wot0_d[:])
            nc.sync.dma_start(out=wot1[:], in_=wot1_d[:])
            nc.sync.dma_start(out=ident[:], in_=ident_d[:])
            nc.sync.dma_start(out=ones64[DH : DH + 1, :], in_=ones64_d[:])
            nc.sync.dma_start(out=ones1[:], in_=ones1_d[:])
            nc.sync.dma_start(out=ones2[:], in_=ones2_d[:])
            if has_bias:
                bqc = stat.tile([C, 1], F32, tag="bqc")
                bkc = stat.tile([C, 1], F32, tag="bkc")
                bop = stat.tile([1, C], BF16, tag="bop")
                nc.sync.dma_start(out=bqc[:], in_=bq_d[:])
                nc.sync.dma_start(out=bkc[:], in_=bk_d[:])
                nc.sync.dma_start(out=bop[:], in_=bop_d[:])

            # per-iteration state handed between pipeline stages
            st = [dict() for _ in range(n_pairs)]

            def s0_load(n):
                s = st[n]
                s["xq"] = xin.tile([128, 3 * C], F32R, tag="xq", name=f"xq{n}")
                s["xkv"] = xin.tile([128, 3 * C], F32R, tag="xkv", name=f"xkv{n}")
                nc.sync.dma_start(
                    out=s["xq"][:].rearrange("p (a c) -> p a c", a=3),
                    in_=xq_d[n].rearrange("(a p) c -> p a c", p=128),
                )
                nc.sync.dma_start(
                    out=s["xkv"][:].rearrange("p (a c) -> p a c", a=3),
                    in_=xkv_d[n].rearrange("(a p) c -> p a c", p=128),
                )

            def s1_transpose(n):
                s = st[n]
                tq = tps.tile([C, NF], F32R, tag="tp", name=f"tq{n}")
                tkv = tps.tile([C, NF], F32R, tag="tp", name=f"tkv{n}")
                for a in range(3):
                    sl = slice(a * 128, (a + 1) * 128)
                    nc.tensor.transpose(tq[:, sl], s["xq"][:, sl], ident[:])
                    nc.tensor.transpose(tkv[:, sl], s["xkv"][:, sl], ident[:])
                s["xtq"] = xtp.tile([C, NF], BF16, tag="xtq", name=f"xtq{n}")
                s["xtkv"] = xtp.tile([C, NF], BF16, tag="xtkv", name=f"xtkv{n}")
                nc.vector.tensor_copy(s["xtq"][:], tq[:].bitcast(F32))
                nc.scalar.copy(s["xtkv"][:], tkv[:].bitcast(F32))

            def s2_proj(n):
                s = st[n]
                qps = pjp.tile([C, NF], F32, tag="pj", name=f"qps{n}")
                nc.tensor.matmul(qps[:], wqt[:], s["xtq"][:], start=True, stop=True)
                s["qt"] = qkp.tile([C, NF], F32R, tag="qt", name=f"qt{n}")
                if has_bias:
                    with nc.allow_low_precision(reason="f32r rounding of qT"):
                        nc.scalar.activation(
                            s["qt"][:], qps[:], AF.Identity, bias=bqc[:], scale=1.0
                        )
                else:
                    nc.vector.tensor_copy(s["qt"][:], qps[:])

                kps = pjp.tile([C, NF], F32, tag="pj", name=f"kps{n}")
                nc.tensor.matmul(kps[:], wkt[:], s["xtkv"][:], start=True, stop=True)
                s["kt"] = qkp.tile([C, NF], F32R, tag="kt", name=f"kt{n}")
                if has_bias:
                    with nc.allow_low_precision(reason="f32r rounding of kT"):
                        nc.scalar.activation(
                            s["kt"][:], kps[:], AF.Identity, bias=bkc[:], scale=1.0
                        )
                else:
                    nc.vector.tensor_copy(s["kt"][:], kps[:])

                # v: 3 chunk matmuls + rank-1 ones into one [128, 390] psum
                vps = pjp.tile([128, 3 * 130], F32, tag="pj", name=f"vps{n}")
                for a in range(3):
                    o = a * 130
                    nc.tensor.matmul(
                        vps[:, o : o + 128],
                        s["xtkv"][:, a * 128 : (a + 1) * 128],
                        wvt[:],
                        start=True,
                        stop=True,
                    )
                    nc.tensor.matmul(
                        vps[:, o + 128 : o + 130], ones1[:], ones2[:],
                        start=True, stop=True,
                    )
                s["v"] = vexp.tile([128, 3 * 130], BF16, tag="v", name=f"v{n}")
                # [p, a, h*65 + d] <- psum[p, a, h*64 + d]
                nc.vector.tensor_copy(
                    s["v"][:]
                    .rearrange("p (a x) -> p a x", a=3)[:, :, 0:130]
                    .rearrange("p a (h x) -> p a h x", h=2)[:, :, :, 0:DH],
                    vps[:]
                    .rearrange("p (a x) -> p a x", a=3)[:, :, 0:128]
                    .rearrange("p a (h d) -> p a h d", h=2),
                )
                nc.vector.tensor_copy(
                    s["v"][:]
                    .rearrange("p (a x) -> p a x", a=3)[:, :, 0:130]
                    .rearrange("p a (h x) -> p a h x", h=2)[:, :, :, DH : DH + 1],
                    vps[:]
                    .rearrange("p (a x) -> p a x", a=3)[:, :, 128:130]
                    .rearrange("p a (h o) -> p a h o", h=2),
                )

            def s3_attention(n):
                s = st[n]
                s["osb"] = outp.tile([DH, 2 * NF], BF16, tag="on", name=f"on{n}")
                atsb = [None, None]
                for h in range(H):
                    hs = h * DH
                    esb = vexp.tile([128, 3 * NF], BF16, tag="exp", name=f"e{n}_{h}")
                    for a in range(3):
                        scps = scp.tile([128, NF], F32, tag="sc", name=f"sc{n}_{h}{a}")
                        nc.tensor.matmul(
                            scps[:],
                            s["kt"][hs : hs + DH, a * 128 : (a + 1) * 128],
                            s["qt"][hs : hs + DH, :],
                            start=True,
                            stop=True,
                        )
                        nc.scalar.activation(
                            esb[:, a * NF : (a + 1) * NF], scps[:], AF.Exp, scale=1.0
                        )
                    at = atp.tile([DH + 1, NF], F32, tag="at", name=f"at{n}_{h}")
                    for a in range(3):
                        nc.tensor.matmul(
                            at[:],
                            s["v"][:, a * 130 + h * 65 : a * 130 + (h + 1) * 65],
                            esb[:, a * NF : (a + 1) * NF],
                            start=(a == 0),
                            stop=(a == 2),
                        )
                    # free the PSUM accumulator: copy (ACT) and recip (DVE)
                    # both read it immediately and in parallel
                    un = smallp.tile([DH, NF], F32, tag="un", name=f"un{n}_{h}")
                    nc.scalar.copy(un[:], at[0:DH, :])
                    rc = smallp.tile([DH + 1, NF], F32R, tag="rc", name=f"rc{n}_{h}")
                    with nc.allow_low_precision(reason="softmax denom recip"):
                        nc.vector.reciprocal(rc[DH : DH + 1, :], at[DH : DH + 1, :])
                    atsb[h] = (un, rc)
                for h in range(H):
                    un, rc = atsb[h]
                    bc = atp.tile([DH, NF], F32, tag="at", name=f"bc{n}_{h}")
                    nc.tensor.matmul(
                        bc[:], ones64[DH : DH + 1, :], rc[DH : DH + 1, :],
                        start=True, stop=True,
                    )
                    nc.vector.tensor_tensor(
                        s["osb"][:, h * NF : (h + 1) * NF], un[:], bc[:], op=MUL
                    )

            def s4_final(n):
                s = st[n]
                fps = tps.tile([128, 3 * C], F32, tag="tp", name=f"fps{n}")
                for a in range(3):
                    sl = slice(a * 128, (a + 1) * 128)
                    nc.tensor.matmul(
                        fps[:, sl],
                        s["osb"][:, 0 * NF + a * 128 : 0 * NF + (a + 1) * 128],
                        wot0[:],
                        start=True,
                        stop=False,
                    )
                    nc.tensor.matmul(
                        fps[:, sl],
                        s["osb"][:, 1 * NF + a * 128 : 1 * NF + (a + 1) * 128],
                        wot1[:],
                        start=False,
                        stop=not has_bias,
                    )
                    if has_bias:
                        nc.tensor.matmul(
                            fps[:, sl], ones1[:], bop[:], start=False, stop=True
                        )
                fout = outp.tile([128, 3 * C], F32, tag="fout", name=f"fo{n}")
                nc.vector.tensor_copy(fout[:], fps[:])
                nc.sync.dma_start(
                    out=out_d[n].rearrange("(a p) c -> p a c", p=128),
                    in_=fout[:].rearrange("p (a c) -> p a c", a=3),
                )
                st[n] = None  # release references

            # software-pipelined emission, oldest stage first
            stages = [s0_load, s1_transpose, s2_proj, s3_attention, s4_final]
            NS = len(stages)

            def emit_all():
                for i in range(n_pairs):
                    st[i] = dict()
                for step in range(n_pairs + NS - 1):
                    for k in range(NS - 1, -1, -1):
                        i = step - k
                        if 0 <= i < n_pairs:
                            stages[k](i)

            if repeat == 1:
                emit_all()
            else:
                with tc.For_i(0, repeat, 1):
                    emit_all()

    nc.finalize()
    return nc


def _get_nc(has_bias, n_pairs=PER_CORE, repeat=1):
    key = ("nc", has_bias, n_pairs, repeat)
    if key not in _CACHE:
        _CACHE[key] = _build(has_bias, n_pairs, repeat)
    return _CACHE[key]


def kernel(q_in, kv_in, Wq, bq, Wk, bk, Wv, bv, Wo, bo):
    import ml_dtypes
    from concourse.bass_utils import run_bass_kernel_spmd

    q_in = np.asarray(q_in, dtype=np.float32)
    kv_in = np.asarray(kv_in, dtype=np.float32)
    Wq = np.asarray(Wq, dtype=np.float32)
    Wk = np.asarray(Wk, dtype=np.float32)
    Wv = np.asarray(Wv, dtype=np.float32)
    Wo = np.asarray(Wo, dtype=np.float32)
    bq = np.asarray(bq, dtype=np.float32)
    bk = np.asarray(bk, dtype=np.float32)
    bv = np.asarray(bv, dtype=np.float32)
    bo = np.asarray(bo, dtype=np.float32)

    bf16 = ml_dtypes.bfloat16
    # fold 1/sqrt(dh) into Wq/bq; fold bv through softmax (rows sum to 1)
    # and Wo into the output bias: out = attn@(v0 + 1 bv) @ Wo.T + bo
    #                                  = attn@v0 @ Wo.T + 1 (Wo bv + bo).
    wqt = (np.ascontiguousarray(Wq.T) * np.float32(SCALE)).astype(bf16)
    wkt = np.ascontiguousarray(Wk.T).astype(bf16)
    wvt = np.ascontiguousarray(Wv.T).astype(bf16)
    wot = Wo.T  # [c, c']
    wot0 = np.ascontiguousarray(wot[0:DH, :]).astype(bf16)
    wot1 = np.ascontiguousarray(wot[DH:C, :]).astype(bf16)
    bqs = (bq * np.float32(SCALE)).reshape(C, 1)
    bks = bk.reshape(C, 1)
    bop = (Wo @ bv + bo).reshape(1, C)
    has_bias = bool(np.any(bqs) or np.any(bks) or np.any(bop))

    nc = _get_nc(has_bias)

    qf = q_in.reshape(PAIRS, NF, C)
    kf = kv_in.reshape(PAIRS, NF, C)

    common = {
        "wqt": wqt,
        "wkt": wkt,
        "wvt": wvt,
        "wot0": wot0,
        "wot1": wot1,
        "ident": np.eye(C, dtype=np.float32),
        "ones64": np.ones((1, DH), dtype=np.float32),
        "ones1": np.ones((1, C), dtype=bf16),
        "ones2": np.ones((1, 2), dtype=bf16),
    }
    if has_bias:
        common["bqc"] = bqs
        common["bkc"] = bks
        common["bop"] = bop.astype(bf16)

    in_maps = []
    for i in range(NCORES):
        m = dict(common)
        m["xq"] = np.ascontiguousarray(qf[i * PER_CORE : (i + 1) * PER_CORE])
        m["xkv"] = np.ascontiguousarray(kf[i * PER_CORE : (i + 1) * PER_CORE])
        in_maps.append(m)

    res = run_bass_kernel_spmd(nc, in_maps, list(range(NCORES)))
    out = np.concatenate([res.results[i]["out"] for i in range(NCORES)], axis=0)
    return out.reshape(B, T, NF, C)
